# revision 1
# baseline (speedup 1.0000x reference)
"""Contrastive loss (CLIP-style, 2 views) on 8 Trainium2 NeuronCores.

Math: with Af/Bf the L2-normalized (V*N, D) view-major matrices,
  loss = mean_i [ logsumexp_{j != i}(Af@Bf.T / T)[i, :] - (Af@Bf.T)[i, p(i)]/T ]
where p(i) = (i + N) mod (V*N) is the other-view partner of row i.
The reference's mask/gather/sort is cosmetic: log_softmax is permutation
invariant, so only "drop the diagonal" and "read the partner column" matter.

Sharding: rows of Af are split across 8 cores (1024 rows each); every core
gets the full B (D-major) with its columns rotated by 1024*k so that the
diagonal of core k's slab lands at *static* local columns (row-chunk m ->
cols [128m, 128m+128) of column-group 0) and the partner diagonal at the
same offset of column-group 2.  This keeps the SPMD program identical on
all cores.  A's per-row 1/(|a|*T) is folded into the Exp activation scale,
so A itself is never normalized on-chip; B is normalized in place (square,
ones-matmul partition-reduce -> broadcast ss, sqrt, approx-reciprocal, mul).

Per core: 256-wide contraction split into 2 K-chunks; logits computed in
(128 x 2048) PSUM tiles (4 banks) with bf16 matmuls (1 row/cycle; operands
rounded to bf16 only after fp32 normalization), fused Exp+row-accumulate on
ACT, diagonal masked additively with -1e9, partner extracted with an
identity-mask multiply + row reduce.  Output is a (128, 1) per-partition
partial sum of (LSE_i - pos_i/T); the host adds the 8*128 partials and
divides by 8192.
"""

import os

import numpy as np

N = 4096
V = 2
D = 256
M = V * N            # 8192 rows/cols of the logits matrix
TEMP = 0.07
NCORES = 8
ROWS = M // NCORES   # 1024 rows per core
P = 128              # partitions
NM = ROWS // P       # 8 row-chunks per core
GW = 2048            # column-group width (one B DMA/normalize unit)
NG = M // GW         # 4 column groups
PSW = int(os.environ.get("KERNEL_PSW", "2048"))  # PSUM tile width
PBUFS = 4096 // PSW  # use all 8 PSUM banks: 2048 -> 2 bufs, 1024 -> 4
NSUB = GW // PSW     # PSUM tiles per column group
NST = M // PSW       # exp accumulator columns per row-chunk
KC = D // P          # 2 contraction chunks
NEG = -1.0e9         # additive mask for the diagonal
# bf16 default: fp8 DoubleRow halves PE time but the kernel is ACT-bound
# (measured identical wall time), so bf16's ~170x better accuracy is free
USE_FP8 = os.environ.get("KERNEL_FP8", "0") != "0"

_CACHE: dict = {}


def _build_nc():
    import concourse.bacc as bacc
    import concourse.bass as bass
    import concourse.mybir as mybir
    import concourse.tile as tile

    f32 = mybir.dt.float32
    bf16 = mybir.dt.bfloat16
    mmdt = mybir.dt.float8e4 if USE_FP8 else bf16
    mm_kwargs = (
        {"perf_mode": mybir.MatmulPerfMode.DoubleRow} if USE_FP8 else {})
    AX = mybir.AxisListType
    OP = mybir.AluOpType
    AF = mybir.ActivationFunctionType

    nc = bacc.Bacc("TRN2", target_bir_lowering=False, debug=False,
                   num_devices=NCORES)

    at_d = nc.dram_tensor("at", (D, ROWS), f32, kind="ExternalInput")
    arow_d = nc.dram_tensor("arow", (ROWS, D), f32, kind="ExternalInput")
    bt_d = nc.dram_tensor("bt", (D, M), f32, kind="ExternalInput")
    dmask_d = nc.dram_tensor("dmask", (P, P), f32, kind="ExternalInput")
    i128_d = nc.dram_tensor("i128", (P, P), f32, kind="ExternalInput")
    out_d = nc.dram_tensor("partials", (P, 1), f32, kind="ExternalOutput")

    with tile.TileContext(nc) as tc:
        with (
            tc.tile_pool(name="big", bufs=1) as big,
            tc.tile_pool(name="work", bufs=2) as work,
            tc.tile_pool(name="psum", bufs=2, space=bass.MemorySpace.PSUM) as pp,
        ):
            # --- persistent SBUF tensors -------------------------------
            at_s = big.tile((P, KC, ROWS), f32)     # A slab, D-major, fp32
            at_b = big.tile((P, KC, ROWS), mmdt)    # A slab (matmul lhsT)
            arow_s = big.tile((P, NM, D), f32)      # A slab, row-major
            bt_b = big.tile((P, KC, M), mmdt)       # normalized B (matmul rhs)
            dmask_s = big.tile((P, P), f32)
            i128_s = big.tile((P, P), f32)
            ones_s = big.tile((P, P), bf16)
            ssa_s = big.tile((P, NM), f32)          # sum(a^2) per slab row
            sqa_s = big.tile((P, NM), f32)
            sca_s = big.tile((P, NM), f32)          # 1/(|a|*T) per slab row
            acc_s = big.tile((P, NM * NST), f32)    # exp row-sums
            praw_s = big.tile((P, NM), f32)         # raw partner dots
            ssum_s = big.tile((P, NM), f32)
            lns_s = big.tile((P, NM), f32)
            lt_s = big.tile((P, NM), f32)
            outp_s = big.tile((P, 1), f32)

            # B group DMAs go first so group 0 lands as early as possible;
            # each dma_start is striped over all 16 DMA engines by the DGE
            btf_tiles = []
            for g in range(NG):
                gsl = slice(g * GW, (g + 1) * GW)
                btf = work.tile((P, KC, GW), f32, tag="btf", bufs=4)
                btf_tiles.append(btf)
                for kc in range(KC):
                    nc.sync.dma_start(
                        btf[:, kc, :],
                        bt_d.ap()[kc * P : (kc + 1) * P, gsl])
            nc.sync.dma_start(
                at_s[:], at_d.ap().rearrange("(k p) r -> p k r", p=P))
            nc.sync.dma_start(
                arow_s[:], arow_d.ap().rearrange("(t p) d -> p t d", p=P))
            nc.sync.dma_start(dmask_s[:], dmask_d.ap())
            nc.sync.dma_start(i128_s[:], i128_d.ap())
            nc.vector.memset(ones_s[:], 1.0)
            # off DVE's and ACT's critical paths (DVE gates the first
            # B-norm square, ACT the exps)
            nc.gpsimd.tensor_copy(at_b[:], at_s[:])

            # --- A row scales: 1 / (|a_i| * T) -------------------------
            # (tensor_tensor_reduce hard-faults the exec unit on this HW
            # path, so square and reduce are separate instructions)
            for m in range(NM):
                asq = work.tile((P, D), f32, tag="asq")
                nc.vector.tensor_mul(asq[:], arow_s[:, m, :], arow_s[:, m, :])
                nc.vector.reduce_sum(ssa_s[:, m : m + 1], asq[:], axis=AX.X)
            nc.scalar.sqrt(sqa_s[:], ssa_s[:])
            nc.vector.reciprocal_approx_fast(out=sca_s[:], in_=sqa_s[:])
            nc.vector.tensor_scalar_mul(sca_s[:], sca_s[:], 1.0 / TEMP)

            # --- phase 0: normalize all of B (keeps ACT tables stable:
            # all Sqrt here, all Exp later) ------------------------------
            for g in range(NG):
                btf = btf_tiles[g]
                for sub in range(NSUB):
                    ssl = slice(sub * PSW, (sub + 1) * PSW)
                    osl = slice(g * GW + sub * PSW, g * GW + (sub + 1) * PSW)
                    # B norms: ss broadcast over partitions via ones-matmul
                    ssb = pp.tile((P, PSW), f32, tag="ps", bufs=PBUFS)
                    for kc in range(KC):
                        bsq = work.tile((P, PSW), bf16, tag="bsq")
                        nc.vector.tensor_mul(bsq[:], btf[:, kc, ssl],
                                             btf[:, kc, ssl])
                        for c in range(PSW // 512):
                            csl = slice(c * 512, (c + 1) * 512)
                            nc.tensor.matmul(
                                ssb[:, csl],
                                ones_s[:],
                                bsq[:, csl],
                                start=(kc == 0), stop=(kc == KC - 1))
                    sqb = work.tile((P, PSW), f32, tag="sqb")
                    nc.scalar.sqrt(sqb[:], ssb[:])
                    invb = work.tile((P, PSW), f32, tag="invb")
                    nc.vector.reciprocal_approx_fast(out=invb[:], in_=sqb[:])
                    for kc in range(KC):
                        # normalize in fp32, rounding only on the write
                        nc.vector.tensor_mul(bt_b[:, kc, osl],
                                             btf[:, kc, ssl], invb[:])

            # --- phase 1: logits + exp row-sums ------------------------
            for g in range(NG):
                for m in range(NM):
                    dsub = (m * P) // PSW  # sub-tile holding the diagonal
                    for sub in range(NSUB):
                        lg = pp.tile((P, PSW), f32, tag="ps", bufs=PBUFS)
                        base = g * GW + sub * PSW
                        if USE_FP8:
                            # DoubleRow: both K-halves in one matmul via
                            # the 3D [128, 2, N] APs
                            for c in range(PSW // 512):
                                csl = slice(c * 512, (c + 1) * 512)
                                bsl = slice(base + c * 512,
                                            base + (c + 1) * 512)
                                nc.tensor.matmul(
                                    lg[:, csl],
                                    at_b[:, :, m * P : (m + 1) * P],
                                    bt_b[:, :, bsl],
                                    start=True, stop=True, **mm_kwargs)
                        else:
                            # kc outer: each A weight tile streams all banks
                            for kc in range(KC):
                                for c in range(PSW // 512):
                                    csl = slice(c * 512, (c + 1) * 512)
                                    bsl = slice(base + c * 512,
                                                base + (c + 1) * 512)
                                    nc.tensor.matmul(
                                        lg[:, csl],
                                        at_b[:, kc, m * P : (m + 1) * P],
                                        bt_b[:, kc, bsl],
                                        start=(kc == 0),
                                        stop=(kc == KC - 1),
                                        skip_group_check=True)
                        if sub == dsub:
                            msl = slice(m * P - dsub * PSW,
                                        m * P - dsub * PSW + P)
                            if g == 0:
                                # additive -1e9 on the diagonal -> exp == 0
                                nc.vector.tensor_add(lg[:, msl], lg[:, msl],
                                                     dmask_s[:])
                            if g == 2:
                                # partner (positive) dot on this diagonal
                                pscr = work.tile((P, P), f32, tag="pscr")
                                nc.vector.tensor_mul(pscr[:], lg[:, msl],
                                                     i128_s[:])
                                nc.vector.reduce_sum(
                                    praw_s[:, m : m + 1], pscr[:], axis=AX.X)
                        esc = work.tile((P, PSW), f32, tag="esc")
                        ai = m * NST + g * NSUB + sub
                        nc.scalar.activation(
                            esc[:], lg[:], AF.Exp,
                            bias=0.0, scale=sca_s[:, m : m + 1],
                            accum_out=acc_s[:, ai : ai + 1])

            # --- assembly: loss rows = ln(S) - praw * sca --------------
            for m in range(NM):
                nc.vector.reduce_sum(
                    ssum_s[:, m : m + 1], acc_s[:, m * NST : (m + 1) * NST],
                    axis=AX.X)
            nc.scalar.activation(lns_s[:], ssum_s[:], AF.Ln)
            nc.vector.tensor_mul(praw_s[:], praw_s[:], sca_s[:])
            nc.vector.tensor_sub(lt_s[:], lns_s[:], praw_s[:])
            nc.vector.reduce_sum(outp_s[:], lt_s[:], axis=AX.X)
            nc.sync.dma_start(out_d.ap(), outp_s[:])

    nc.compile()
    return nc


def get_nc():
    if "nc" not in _CACHE:
        _CACHE["nc"] = _build_nc()
    return _CACHE["nc"]


def make_in_maps(A: np.ndarray, B: np.ndarray) -> list[dict]:
    A = np.asarray(A, dtype=np.float32)
    B = np.asarray(B, dtype=np.float32)
    # view-major D-major matrices: X[d, v*N + n] = X_in[n, v, d]
    At = np.ascontiguousarray(A.transpose(2, 1, 0).reshape(D, M))
    Bt = np.ascontiguousarray(B.transpose(2, 1, 0).reshape(D, M))
    dmask = np.zeros((P, P), dtype=np.float32)
    np.fill_diagonal(dmask, NEG)
    i128 = np.eye(P, dtype=np.float32)
    in_maps = []
    for k in range(NCORES):
        at_k = np.ascontiguousarray(At[:, k * ROWS : (k + 1) * ROWS])
        arow_k = np.ascontiguousarray(at_k.T)
        # rotate columns so local col j holds global col (j + 1024k) % 8192
        bt_k = np.ascontiguousarray(np.roll(Bt, -ROWS * k, axis=1))
        in_maps.append({"at": at_k, "arow": arow_k, "bt": bt_k,
                        "dmask": dmask, "i128": i128})
    return in_maps


def kernel(A: np.ndarray, B: np.ndarray) -> np.ndarray:
    from concourse.bass_utils import run_bass_kernel_spmd

    in_maps = make_in_maps(A, B)
    nc = get_nc()
    trace = bool(int(os.environ.get("KERNEL_TRACE", "0")))
    res = run_bass_kernel_spmd(
        nc, in_maps, core_ids=list(range(NCORES)), trace=trace)
    total = 0.0
    for r in res.results:
        total += float(r["partials"].astype(np.float64).sum())
    if res.exec_time_ns is not None:
        print(f"[kernel] exec_time_ns={res.exec_time_ns}")
        _CACHE["exec_time_ns"] = res.exec_time_ns
    _CACHE["last_results"] = res
    return np.float32(total / M)



# revision 6
# speedup vs baseline: 1.1085x; 1.1085x over previous
"""Contrastive loss (CLIP-style, 2 views) on 8 Trainium2 NeuronCores.

Math: with Af/Bf the L2-normalized (V*N, D) view-major matrices,
  loss = mean_i [ logsumexp_{j != i}(Af@Bf.T / T)[i, :] - (Af@Bf.T)[i, p(i)]/T ]
where p(i) = (i + N) mod (V*N) is the other-view partner of row i.
The reference's mask/gather/sort is cosmetic: log_softmax is permutation
invariant, so only "drop the diagonal" and "read the partner column" matter.

Sharding: rows of Af are split across 8 cores (1024 rows each); every core
gets the full B (D-major) with its columns rotated by 1024*k so that the
diagonal of core k's slab lands at *static* local columns (row-chunk m ->
cols [128m, 128m+128) of column-group 0) and the partner diagonal at the
same offset of column-group 2.  This keeps the SPMD program identical on
all cores.  A's per-row 1/(|a|*T) is folded into the Exp activation scale.

Pipelined structure (the whole point of this version): the B-normalize of
column-group g+1 is emitted *inside* group g's logits loop, so normalize,
matmul and exp all overlap and the ACT engine (the bottleneck: 32 exp
tiles of (128,2048) ~2us each) streams without phase gaps.  All ACT
functions are from one table set (Exp/Ln: rsqrt(x) = Exp(-0.5*Ln(x))), so
there are zero 1283ns table reloads.  Logits matmuls run in fp8e4 with
DoubleRow (both 128-deep K-chunks in one pass); the diagonal -1e9 mask is
a bf16 identity matmul appended to the same PSUM accumulation group; exp
is computed in-place in PSUM (PSUM access is cheaper for ACT than SBUF).
"""

import os

import numpy as np

N = 4096
V = 2
D = 256
M = V * N            # 8192 rows/cols of the logits matrix
TEMP = 0.07
NCORES = 8
ROWS = M // NCORES   # 1024 rows per core
P = 128              # partitions
NM = ROWS // P       # 8 row-chunks per core
GW = 2048            # column-group width (one B normalize unit)
NG = M // GW         # 4 column groups
PSW = 2048           # PSUM tile width (4 banks; ring of 2 = all 8 banks)
NSUB = GW // PSW     # PSUM tiles per column group (1)
NST = M // PSW       # exp accumulator columns per row-chunk (4)
KC = D // P          # 2 contraction chunks
NEG = -1.0e9         # additive mask for the diagonal
# fp8 DoubleRow halves PE time; accuracy ~2e-4 rel on the final scalar
# (noise averages over 8192 rows), far inside the 2e-2 gate.
USE_FP8 = os.environ.get("KERNEL_FP8", "1") != "0"

_CACHE: dict = {}


def _build_nc():
    import math

    import concourse.bacc as bacc
    import concourse.bass as bass
    import concourse.mybir as mybir
    import concourse.tile as tile

    f32 = mybir.dt.float32
    bf16 = mybir.dt.bfloat16
    mmdt = mybir.dt.float8e4 if USE_FP8 else bf16
    AX = mybir.AxisListType
    AF = mybir.ActivationFunctionType

    nc = bacc.Bacc("TRN2", target_bir_lowering=False, debug=False,
                   num_devices=NCORES)

    at_d = nc.dram_tensor("at", (D, ROWS), f32, kind="ExternalInput")
    arow_d = nc.dram_tensor("arow", (ROWS, D), f32, kind="ExternalInput")
    bt_d = nc.dram_tensor("bt", (D, M), f32, kind="ExternalInput")
    dmask_d = nc.dram_tensor("dmask", (P, P), bf16, kind="ExternalInput")
    i128_d = nc.dram_tensor("i128", (P, P), bf16, kind="ExternalInput")
    i128f_d = nc.dram_tensor("i128f", (P, P), f32, kind="ExternalInput")
    out_d = nc.dram_tensor("partials", (P, 1), f32, kind="ExternalOutput")

    with tile.TileContext(nc) as tc:
        with (
            tc.tile_pool(name="big", bufs=1) as big,
            tc.tile_pool(name="work", bufs=2) as work,
            tc.tile_pool(name="psum", bufs=2, space=bass.MemorySpace.PSUM) as pp,
        ):
            # --- persistent SBUF tensors -------------------------------
            at_s = big.tile((P, KC, ROWS), f32)     # A slab, D-major, fp32
            at_b = big.tile((P, KC, ROWS), mmdt)    # A slab (matmul lhsT)
            arow_s = big.tile((P, NM, D), f32)      # A slab, row-major
            bt_b = big.tile((P, KC, M), mmdt)       # normalized B (rhs)
            dmask_s = big.tile((P, P), bf16)        # -1e9 * I  (mask weights)
            i128_s = big.tile((P, P), bf16)         # identity (mask rhs)
            i128f_s = big.tile((P, P), f32)         # identity (praw extract)
            ones_s = big.tile((P, P), bf16)
            ssa_s = big.tile((P, NM), f32)          # sum(a^2) per slab row
            sca_s = big.tile((P, NM), f32)          # 1/(|a|*T) per slab row
            acc_s = big.tile((P, NM * NST), f32)    # exp row-sums
            praw_s = big.tile((P, NM), f32)         # raw partner dots
            ssum_s = big.tile((P, NM), f32)
            lns_s = big.tile((P, NM), f32)
            lt_s = big.tile((P, NM), f32)
            outp_s = big.tile((P, 1), f32)

            # --- DMAs: B group 0 first (it gates the pipeline fill),
            # then A (gates first matmul + exp scale), then groups 1-3.
            btf_tiles = [None] * NG

            def dma_group(g):
                gsl = slice(g * GW, (g + 1) * GW)
                btf = work.tile((P, KC, GW), f32, tag="btf", bufs=4)
                btf_tiles[g] = btf
                for kc in range(KC):
                    nc.sync.dma_start(
                        btf[:, kc, :],
                        bt_d.ap()[kc * P : (kc + 1) * P, gsl])

            dma_group(0)
            nc.sync.dma_start(
                at_s[:], at_d.ap().rearrange("(k p) r -> p k r", p=P))
            nc.sync.dma_start(
                arow_s[:], arow_d.ap().rearrange("(t p) d -> p t d", p=P))
            nc.sync.dma_start(dmask_s[:], dmask_d.ap())
            nc.sync.dma_start(i128_s[:], i128_d.ap())
            nc.sync.dma_start(i128f_s[:], i128f_d.ap())
            for g in range(1, NG):
                dma_group(g)
            nc.vector.memset(ones_s[:], 1.0)

            # at_b cast on GpSimd: off DVE's critical path (DVE gates the
            # first B-norm square and normalize-mul)
            nc.gpsimd.tensor_copy(at_b[:], at_s[:])

            def emit_norm(g):
                """B-normalize column group g: square (DVE) -> partition
                reduce (PE ones-matmul -> PSUM) -> rsqrt = Exp(-.5*Ln)
                (ACT, no table switch) -> scale columns (DVE, fp8 out)."""
                btf = btf_tiles[g]
                osl = slice(g * GW, (g + 1) * GW)
                ssb = pp.tile((P, GW), f32, tag="ps", bufs=2)
                for kc in range(KC):
                    bsq = work.tile((P, GW), bf16, tag="bsq")
                    nc.vector.tensor_mul(bsq[:], btf[:, kc, :], btf[:, kc, :])
                    for c in range(GW // 512):
                        csl = slice(c * 512, (c + 1) * 512)
                        nc.tensor.matmul(
                            ssb[:, csl], ones_s[:], bsq[:, csl],
                            start=(kc == 0), stop=(kc == KC - 1),
                            skip_group_check=True)
                invb = work.tile((P, GW), f32, tag="invb")
                nc.scalar.activation(invb[:], ssb[:], AF.Ln)
                nc.scalar.activation(invb[:], invb[:], AF.Exp, scale=-0.5)
                for kc in range(KC):
                    nc.vector.tensor_mul(bt_b[:, kc, osl],
                                         btf[:, kc, :], invb[:])

            # --- pipeline fill: normalize group 0, compute A row scales
            emit_norm(0)

            # A row scales: sca = 1/(|a|*T) = Exp(-0.5*Ln(ssa) + ln(1/T)).
            # (tensor_tensor_reduce hard-faults the exec unit on this HW
            # path, so square and reduce stay separate instructions)
            asq = work.tile((P, NM, D), f32, tag="asq", bufs=1)
            nc.vector.tensor_mul(asq[:], arow_s[:], arow_s[:])
            for m in range(NM):
                nc.vector.reduce_sum(ssa_s[:, m : m + 1],
                                     asq[:, m, :], axis=AX.X)
            nc.scalar.activation(sca_s[:], ssa_s[:], AF.Ln)
            nc.scalar.activation(sca_s[:], sca_s[:], AF.Exp, scale=-0.5)
            nc.vector.tensor_scalar_mul(sca_s[:], sca_s[:], 1.0 / TEMP)

            # --- main loop: logits + exp for group g; group g+1's
            # normalize is emitted after m==1 so its PSUM ring slot only
            # waits on exp(g, m0) and its result is ready long before
            # group g+1's matmuls need it.
            for g in range(NG):
                for m in range(NM):
                    if m == 2 and g + 1 < NG:
                        emit_norm(g + 1)
                    lg = pp.tile((P, PSW), f32, tag="ps", bufs=2)
                    dchunk = (m * P) // 512  # chunk holding the diagonal
                    if USE_FP8:
                        for c in range(PSW // 512):
                            csl = slice(c * 512, (c + 1) * 512)
                            bsl = slice(g * GW + c * 512,
                                        g * GW + (c + 1) * 512)
                            nc.tensor.matmul(
                                lg[:, csl],
                                at_b[:, :, m * P : (m + 1) * P],
                                bt_b[:, :, bsl],
                                start=True,
                                stop=(g != 0 or c != dchunk),
                                skip_group_check=True,
                                perf_mode=mybir.MatmulPerfMode.DoubleRow)
                    else:
                        for kc in range(KC):
                            for c in range(PSW // 512):
                                csl = slice(c * 512, (c + 1) * 512)
                                bsl = slice(g * GW + c * 512,
                                            g * GW + (c + 1) * 512)
                                nc.tensor.matmul(
                                    lg[:, csl],
                                    at_b[:, kc, m * P : (m + 1) * P],
                                    bt_b[:, kc, bsl],
                                    start=(kc == 0),
                                    stop=(kc == KC - 1)
                                    and (g != 0 or c != dchunk),
                                    skip_group_check=True)
                    msl = slice(m * P, m * P + P)
                    if g == 0:
                        # diagonal -1e9: one more matmul into the same
                        # accumulation group (dmask_s = -1e9 * I weights,
                        # identity rhs adds -1e9*I to the diagonal block)
                        nc.tensor.matmul(
                            lg[:, msl], dmask_s[:], i128_s[:],
                            start=False, stop=True, skip_group_check=True)
                    if g == 2:
                        # partner (positive) dot from this diagonal block
                        pscr = work.tile((P, P), f32, tag="pscr")
                        nc.vector.tensor_mul(pscr[:], lg[:, msl], i128f_s[:])
                        nc.vector.reduce_sum(
                            praw_s[:, m : m + 1], pscr[:], axis=AX.X)
                    ai = m * NST + g
                    # exp in place in PSUM; row-sum via the ACT accumulator
                    nc.scalar.activation(
                        lg[:], lg[:], AF.Exp,
                        bias=0.0, scale=sca_s[:, m : m + 1],
                        accum_out=acc_s[:, ai : ai + 1])

            # --- assembly: loss rows = ln(S) - praw * sca --------------
            for m in range(NM):
                nc.vector.reduce_sum(
                    ssum_s[:, m : m + 1], acc_s[:, m * NST : (m + 1) * NST],
                    axis=AX.X)
            nc.scalar.activation(lns_s[:], ssum_s[:], AF.Ln)
            nc.vector.tensor_mul(praw_s[:], praw_s[:], sca_s[:])
            nc.vector.tensor_sub(lt_s[:], lns_s[:], praw_s[:])
            nc.vector.reduce_sum(outp_s[:], lt_s[:], axis=AX.X)
            nc.sync.dma_start(out_d.ap(), outp_s[:])

    nc.compile()
    return nc


def get_nc():
    if "nc" not in _CACHE:
        _CACHE["nc"] = _build_nc()
    return _CACHE["nc"]


def make_in_maps(A: np.ndarray, B: np.ndarray) -> list[dict]:
    A = np.asarray(A, dtype=np.float32)
    B = np.asarray(B, dtype=np.float32)
    # view-major D-major matrices: X[d, v*N + n] = X_in[n, v, d]
    At = np.ascontiguousarray(A.transpose(2, 1, 0).reshape(D, M))
    Bt = np.ascontiguousarray(B.transpose(2, 1, 0).reshape(D, M))
    import ml_dtypes
    dmask = np.zeros((P, P), dtype=np.float32)
    np.fill_diagonal(dmask, NEG)
    dmask = dmask.astype(ml_dtypes.bfloat16)
    i128f = np.eye(P, dtype=np.float32)
    i128 = i128f.astype(ml_dtypes.bfloat16)
    in_maps = []
    for k in range(NCORES):
        at_k = np.ascontiguousarray(At[:, k * ROWS : (k + 1) * ROWS])
        arow_k = np.ascontiguousarray(at_k.T)
        # rotate columns so local col j holds global col (j + 1024k) % 8192
        bt_k = np.ascontiguousarray(np.roll(Bt, -ROWS * k, axis=1))
        in_maps.append({"at": at_k, "arow": arow_k, "bt": bt_k,
                        "dmask": dmask, "i128": i128, "i128f": i128f})
    return in_maps


def kernel(A: np.ndarray, B: np.ndarray) -> np.ndarray:
    from concourse.bass_utils import run_bass_kernel_spmd

    in_maps = make_in_maps(A, B)
    nc = get_nc()
    trace = bool(int(os.environ.get("KERNEL_TRACE", "0")))
    res = run_bass_kernel_spmd(
        nc, in_maps, core_ids=list(range(NCORES)), trace=trace)
    total = 0.0
    for r in res.results:
        total += float(r["partials"].astype(np.float64).sum())
    if res.exec_time_ns is not None:
        print(f"[kernel] exec_time_ns={res.exec_time_ns}")
        _CACHE["exec_time_ns"] = res.exec_time_ns
    _CACHE["last_results"] = res
    return np.float32(total / M)


# revision 11
# speedup vs baseline: 1.1886x; 1.0723x over previous
"""Contrastive loss (CLIP-style, 2 views) on 8 Trainium2 NeuronCores.

Math: with Af/Bf the L2-normalized (V*N, D) view-major matrices,
  loss = mean_i [ logsumexp_{j != i}(Af@Bf.T / T)[i, :] - (Af@Bf.T)[i, p(i)]/T ]
where p(i) = (i + N) mod (V*N) is the other-view partner of row i.
The reference's mask/gather/sort is cosmetic: log_softmax is permutation
invariant, so only "drop the diagonal" and "read the partner column" matter.

Sharding: rows of Af are split across 8 cores (1024 rows each); every core
gets the full B (D-major) with its columns rotated by 1024*k so that the
diagonal of core k's slab lands at *static* local columns (row-chunk m ->
cols [128m, 128m+128) of column-group 0) and the partner diagonal at the
same offset of column-group 2.  This keeps the SPMD program identical on
all cores.  A's per-row 1/(|a|*T) is folded into the Exp activation scale.

Pipelined structure (the whole point of this version): the B-normalize of
column-group g+1 is emitted *inside* group g's logits loop, so normalize,
matmul and exp all overlap and the ACT engine (the bottleneck: 32 exp
tiles of (128,2048) ~2us each) streams without phase gaps.  All ACT
functions are from one table set (Exp/Ln: rsqrt(x) = Exp(-0.5*Ln(x))), so
there are zero 1283ns table reloads.  Logits matmuls run in fp8e4 with
DoubleRow (both 128-deep K-chunks in one pass); the diagonal -1e9 mask is
a bf16 identity matmul appended to the same PSUM accumulation group; exp
is computed in-place in PSUM (PSUM access is cheaper for ACT than SBUF).
"""

import os

import numpy as np

N = 4096
V = 2
D = 256
M = V * N            # 8192 rows/cols of the logits matrix
TEMP = 0.07
NCORES = 8
ROWS = M // NCORES   # 1024 rows per core
P = 128              # partitions
NM = ROWS // P       # 8 row-chunks per core
GW = 2048            # column-group width (one B normalize unit)
NG = M // GW         # 4 column groups
PSW = 2048           # PSUM tile width (4 banks; ring of 2 = all 8 banks)
NSUB = GW // PSW     # PSUM tiles per column group (1)
NST = M // PSW       # exp accumulator columns per row-chunk (4)
KC = D // P          # 2 contraction chunks
NEG = -1.0e9         # additive mask for the diagonal
# fp8 DoubleRow halves PE time; accuracy ~2e-4 rel on the final scalar
# (noise averages over 8192 rows), far inside the 2e-2 gate.
USE_FP8 = os.environ.get("KERNEL_FP8", "1") != "0"

_CACHE: dict = {}


def _build_nc():
    import math

    import concourse.bacc as bacc
    import concourse.bass as bass
    import concourse.mybir as mybir
    import concourse.tile as tile

    f32 = mybir.dt.float32
    bf16 = mybir.dt.bfloat16
    mmdt = mybir.dt.float8e4 if USE_FP8 else bf16
    AX = mybir.AxisListType
    AF = mybir.ActivationFunctionType

    nc = bacc.Bacc("TRN2", target_bir_lowering=False, debug=False,
                   num_devices=NCORES)

    at_d = nc.dram_tensor("at", (D, ROWS), f32, kind="ExternalInput")
    arow_d = nc.dram_tensor("arow", (ROWS, D), f32, kind="ExternalInput")
    bt_d = nc.dram_tensor("bt", (D, M), f32, kind="ExternalInput")
    dmask_d = nc.dram_tensor("dmask", (P, P), bf16, kind="ExternalInput")
    i128_d = nc.dram_tensor("i128", (P, P), bf16, kind="ExternalInput")
    i128f_d = nc.dram_tensor("i128f", (P, P), f32, kind="ExternalInput")
    out_d = nc.dram_tensor("partials", (P, 1), f32, kind="ExternalOutput")

    with tile.TileContext(nc) as tc:
        with (
            tc.tile_pool(name="big", bufs=1) as big,
            tc.tile_pool(name="work", bufs=2) as work,
            tc.tile_pool(name="psum", bufs=2, space=bass.MemorySpace.PSUM) as pp,
        ):
            # --- persistent SBUF tensors -------------------------------
            at_s = big.tile((P, KC, ROWS), f32)     # A slab, D-major, fp32
            at_b = big.tile((P, KC, ROWS), mmdt)    # A slab (matmul lhsT)
            arow_s = big.tile((P, NM, D), f32)      # A slab, row-major
            bt_b = big.tile((P, KC, M), mmdt)       # normalized B (rhs)
            dmask_s = big.tile((P, P), bf16)        # -1e9 * I  (mask weights)
            i128_s = big.tile((P, P), bf16)         # identity (mask rhs)
            i128f_s = big.tile((P, P), f32)         # identity (praw extract)
            ones_s = big.tile((P, P), bf16)
            ssa_s = big.tile((P, NM), f32)          # sum(a^2) per slab row
            sca_s = big.tile((P, NM), f32)          # 1/(|a|*T) per slab row
            lnit_s = big.tile((P, 1), f32)          # ln(1/T) bias vector
            acc_s = big.tile((P, NM * NST), f32)    # exp row-sums
            praw_s = big.tile((P, NM), f32)         # raw partner dots
            ssum_s = big.tile((P, NM), f32)
            lns_s = big.tile((P, NM), f32)
            lt_s = big.tile((P, NM), f32)
            outp_s = big.tile((P, 1), f32)

            # --- DMAs: B group 0 first (it gates the pipeline fill),
            # then A (gates first matmul + exp scale), then groups 1-3.
            btf_tiles = [None] * NG

            def dma_group(g):
                gsl = slice(g * GW, (g + 1) * GW)
                btf = work.tile((P, KC, GW), f32, tag="btf", bufs=4)
                btf_tiles[g] = btf
                for kc in range(KC):
                    nc.sync.dma_start(
                        btf[:, kc, :],
                        bt_d.ap()[kc * P : (kc + 1) * P, gsl])

            dma_group(0)
            nc.sync.dma_start(
                arow_s[:], arow_d.ap().rearrange("(t p) d -> p t d", p=P))
            nc.sync.dma_start(
                at_s[:], at_d.ap().rearrange("(k p) r -> p k r", p=P))
            nc.sync.dma_start(dmask_s[:], dmask_d.ap())
            nc.sync.dma_start(i128_s[:], i128_d.ap())
            nc.sync.dma_start(i128f_s[:], i128f_d.ap())
            for g in range(1, NG):
                dma_group(g)
            nc.vector.memset(ones_s[:], 1.0)
            nc.vector.memset(lnit_s[:], math.log(1.0 / TEMP))

            # Pre-load the ACT table set that holds BOTH Exp and Ln
            # (act_info.json's natural_log_exp_and_others). Every ACT
            # function below comes from this one set, so the fixpoint
            # table-load pass inserts no further 1283ns reloads.
            from concourse.hw_specs import get_activation_tables
            _tabs = list(get_activation_tables(nc.m.arch))
            nc.scalar.add_instruction(
                mybir.InstLoadActFuncSet(
                    name=nc.scalar.bass.get_next_instruction_name(),
                    ins=[], outs=[],
                    act_func_set_id=_tabs.index("natural_log_exp_and_others")))

            # at_b cast on GpSimd: off DVE's critical path. Row-chunk 0
            # first (it gates the very first logits matmul), bulk after.
            nc.gpsimd.tensor_copy(at_b[:, :, 0:P], at_s[:, :, 0:P])
            nc.gpsimd.tensor_copy(at_b[:, :, P:ROWS], at_s[:, :, P:ROWS])

            def emit_norm_squares(g):
                """B-normalize part 1: square (DVE) + partition reduce
                (PE ones-matmul -> PSUM broadcast)."""
                btf = btf_tiles[g]
                ssb = pp.tile((P, GW), f32, tag="ps", bufs=2)
                for kc in range(KC):
                    bsq = work.tile((P, GW), bf16, tag="bsq")
                    nc.vector.tensor_mul(bsq[:], btf[:, kc, :], btf[:, kc, :])
                    for c in range(GW // 512):
                        csl = slice(c * 512, (c + 1) * 512)
                        nc.tensor.matmul(
                            ssb[:, csl], ones_s[:], bsq[:, csl],
                            start=(kc == 0), stop=(kc == KC - 1),
                            skip_group_check=True)
                return ssb

            def emit_norm_scale(g, ssb):
                """B-normalize part 2: rsqrt = Exp(-.5*Ln) on ACT (same
                table as the exps -> no reload) -> scale columns (DVE)."""
                btf = btf_tiles[g]
                osl = slice(g * GW, (g + 1) * GW)
                invb = work.tile((P, GW), f32, tag="invb")
                nc.scalar.activation(invb[:], ssb[:], AF.Ln)
                nc.scalar.activation(invb[:], invb[:], AF.Exp, scale=-0.5)
                for kc in range(KC):
                    nc.vector.tensor_mul(bt_b[:, kc, osl],
                                         btf[:, kc, :], invb[:])

            def emit_norm(g):
                emit_norm_scale(g, emit_norm_squares(g))

            # --- pipeline fill: group 0 squares first (gated only on the
            # B DMA), then the A-scale chain on DVE (gated on arow; runs
            # while ACT does group 0's rsqrt), then group 0's scale.
            ssb0 = emit_norm_squares(0)

            # A row scales: sca = Exp(-0.5*Ln(ssa)) / T = 1/(|a|*T).
            # (tensor_tensor_reduce hard-faults the exec unit on this HW
            # path, so square and reduce stay separate instructions)
            asq = work.tile((P, NM, D), f32, tag="asq", bufs=1)
            nc.vector.tensor_mul(asq[:], arow_s[:], arow_s[:])
            for m in range(NM):
                nc.vector.reduce_sum(ssa_s[:, m : m + 1],
                                     asq[:, m, :], axis=AX.X)
            nc.scalar.activation(sca_s[:], ssa_s[:], AF.Ln)
            nc.scalar.activation(sca_s[:], sca_s[:], AF.Exp,
                                 scale=-0.5, bias=lnit_s[:])

            emit_norm_scale(0, ssb0)

            # --- main loop: logits + exp for group g; group g+1's
            # normalize is emitted after m==1 so its PSUM ring slot only
            # waits on exp(g, m0) and its result is ready long before
            # group g+1's matmuls need it.
            for g in range(NG):
                for m in range(NM):
                    if m == 2 and g + 1 < NG:
                        emit_norm(g + 1)
                    lg = pp.tile((P, PSW), f32, tag="ps", bufs=2)
                    dchunk = (m * P) // 512  # chunk holding the diagonal
                    if USE_FP8:
                        for c in range(PSW // 512):
                            csl = slice(c * 512, (c + 1) * 512)
                            bsl = slice(g * GW + c * 512,
                                        g * GW + (c + 1) * 512)
                            nc.tensor.matmul(
                                lg[:, csl],
                                at_b[:, :, m * P : (m + 1) * P],
                                bt_b[:, :, bsl],
                                start=True,
                                stop=(g != 0 or c != dchunk),
                                skip_group_check=True,
                                perf_mode=mybir.MatmulPerfMode.DoubleRow)
                    else:
                        for kc in range(KC):
                            for c in range(PSW // 512):
                                csl = slice(c * 512, (c + 1) * 512)
                                bsl = slice(g * GW + c * 512,
                                            g * GW + (c + 1) * 512)
                                nc.tensor.matmul(
                                    lg[:, csl],
                                    at_b[:, kc, m * P : (m + 1) * P],
                                    bt_b[:, kc, bsl],
                                    start=(kc == 0),
                                    stop=(kc == KC - 1)
                                    and (g != 0 or c != dchunk),
                                    skip_group_check=True)
                    msl = slice(m * P, m * P + P)
                    if g == 0:
                        # diagonal -1e9: one more matmul into the same
                        # accumulation group (dmask_s = -1e9 * I weights,
                        # identity rhs adds -1e9*I to the diagonal block)
                        nc.tensor.matmul(
                            lg[:, msl], dmask_s[:], i128_s[:],
                            start=False, stop=True, skip_group_check=True)
                    if g == 2:
                        # partner (positive) dot from this diagonal block
                        pscr = work.tile((P, P), f32, tag="pscr")
                        nc.vector.tensor_mul(pscr[:], lg[:, msl], i128f_s[:])
                        nc.vector.reduce_sum(
                            praw_s[:, m : m + 1], pscr[:], axis=AX.X)
                    ai = m * NST + g
                    # exp in place in PSUM; row-sum via the ACT accumulator
                    nc.scalar.activation(
                        lg[:], lg[:], AF.Exp,
                        bias=0.0, scale=sca_s[:, m : m + 1],
                        accum_out=acc_s[:, ai : ai + 1])

            # --- assembly: loss rows = ln(S) - praw * sca --------------
            for m in range(NM):
                nc.vector.reduce_sum(
                    ssum_s[:, m : m + 1], acc_s[:, m * NST : (m + 1) * NST],
                    axis=AX.X)
            nc.scalar.activation(lns_s[:], ssum_s[:], AF.Ln)
            nc.vector.tensor_mul(praw_s[:], praw_s[:], sca_s[:])
            nc.vector.tensor_sub(lt_s[:], lns_s[:], praw_s[:])
            nc.vector.reduce_sum(outp_s[:], lt_s[:], axis=AX.X)
            nc.sync.dma_start(out_d.ap(), outp_s[:])

    nc.compile()
    return nc


def get_nc():
    if "nc" not in _CACHE:
        _CACHE["nc"] = _build_nc()
    return _CACHE["nc"]


def make_in_maps(A: np.ndarray, B: np.ndarray) -> list[dict]:
    A = np.asarray(A, dtype=np.float32)
    B = np.asarray(B, dtype=np.float32)
    # view-major D-major matrices: X[d, v*N + n] = X_in[n, v, d]
    At = np.ascontiguousarray(A.transpose(2, 1, 0).reshape(D, M))
    Bt = np.ascontiguousarray(B.transpose(2, 1, 0).reshape(D, M))
    import ml_dtypes
    dmask = np.zeros((P, P), dtype=np.float32)
    np.fill_diagonal(dmask, NEG)
    dmask = dmask.astype(ml_dtypes.bfloat16)
    i128f = np.eye(P, dtype=np.float32)
    i128 = i128f.astype(ml_dtypes.bfloat16)
    in_maps = []
    for k in range(NCORES):
        at_k = np.ascontiguousarray(At[:, k * ROWS : (k + 1) * ROWS])
        arow_k = np.ascontiguousarray(at_k.T)
        # rotate columns so local col j holds global col (j + 1024k) % 8192
        bt_k = np.ascontiguousarray(np.roll(Bt, -ROWS * k, axis=1))
        in_maps.append({"at": at_k, "arow": arow_k, "bt": bt_k,
                        "dmask": dmask, "i128": i128, "i128f": i128f})
    return in_maps


def kernel(A: np.ndarray, B: np.ndarray) -> np.ndarray:
    from concourse.bass_utils import run_bass_kernel_spmd

    in_maps = make_in_maps(A, B)
    nc = get_nc()
    trace = bool(int(os.environ.get("KERNEL_TRACE", "0")))
    res = run_bass_kernel_spmd(
        nc, in_maps, core_ids=list(range(NCORES)), trace=trace)
    total = 0.0
    for r in res.results:
        total += float(r["partials"].astype(np.float64).sum())
    if res.exec_time_ns is not None:
        print(f"[kernel] exec_time_ns={res.exec_time_ns}")
        _CACHE["exec_time_ns"] = res.exec_time_ns
    _CACHE["last_results"] = res
    return np.float32(total / M)


# revision 18
# speedup vs baseline: 1.2118x; 1.0195x over previous
"""Contrastive loss (CLIP-style, 2 views) on 8 Trainium2 NeuronCores.

Math: with Af/Bf the L2-normalized (V*N, D) view-major matrices,
  loss = mean_i [ logsumexp_{j != i}(Af@Bf.T / T)[i, :] - (Af@Bf.T)[i, p(i)]/T ]
where p(i) = (i + N) mod (V*N) is the other-view partner of row i.
The reference's mask/gather/sort is cosmetic: log_softmax is permutation
invariant, so only "drop the diagonal" and "read the partner column" matter.

Sharding: rows of Af are split across 8 cores (1024 rows each); every core
gets the full B (D-major) with its columns rotated by 1024*k so that the
diagonal of core k's slab lands at *static* local columns (row-chunk m ->
cols [128m, 128m+128) of column-group 0) and the partner diagonal at local
cols [4096+128m, ...) of column-group 2.  This keeps the SPMD program
identical on all cores.  A's 1/(|a|*T) is folded into the Exp scale.

The kernel is ACT(scalar-engine)-bound: 32 exp tiles of (128,2048) at
~2.1us each.  Everything else is arranged to keep ACT streaming:
 - column-group g+1's B-normalize is emitted inside group g's logits loop
   (normalize/matmul/exp fully overlap; the shared 2-slot PSUM ring then
   never stalls the exp stream);
 - ALL ACT functions come from one table set (Exp/Ln, preloaded
   explicitly; rsqrt(x) = Exp(-0.5*Ln(x))) -> zero 1283ns table reloads;
 - the rsqrt runs full-width on ACT (distributed variants need a
   partition broadcast, and both GpSimd partition_broadcast and fp32 PE
   transpose silently produce zeros on this stack);
 - exp is computed in-place in PSUM with the row-sum from the ACT
   accumulator; the diagonal -1e9 mask is one extra bf16 identity matmul
   in the same PSUM accumulation group;
 - the positives come from a separate elementwise product + column-reduce
   (never touching the exp tiles, so ACT never waits on a DVE read);
 - logits matmuls are fp8e4 DoubleRow (both 128-deep K-chunks per pass).
"""

import os

import numpy as np

N = 4096
V = 2
D = 256
M = V * N            # 8192 rows/cols of the logits matrix
TEMP = 0.07
NCORES = 8
ROWS = M // NCORES   # 1024 rows per core
P = 128              # partitions
NM = ROWS // P       # 8 row-chunks per core
GW = 2048            # column-group width (one B normalize unit)
NG = M // GW         # 4 column groups
PSW = 2048           # PSUM tile width (4 banks; ring of 2 = all 8 banks)
NCH = GW // P        # 16 column-norm chunks per group
NST = M // PSW       # exp accumulator columns per row-chunk (4)
KC = D // P          # 2 contraction chunks
NEG = -1.0e9         # additive mask for the diagonal
USE_FP8 = os.environ.get("KERNEL_FP8", "1") != "0"

_CACHE: dict = {}


def _build_nc():
    import math

    import concourse.bacc as bacc
    import concourse.bass as bass
    import concourse.mybir as mybir
    import concourse.tile as tile

    f32 = mybir.dt.float32
    bf16 = mybir.dt.bfloat16
    mmdt = mybir.dt.float8e4 if USE_FP8 else bf16
    AX = mybir.AxisListType
    AF = mybir.ActivationFunctionType

    nc = bacc.Bacc("TRN2", target_bir_lowering=False, debug=False,
                   num_devices=NCORES)

    at_d = nc.dram_tensor("at", (D, ROWS), f32, kind="ExternalInput")
    bt_d = nc.dram_tensor("bt", (D, M), f32, kind="ExternalInput")
    dmask_d = nc.dram_tensor("dmask", (P, P), bf16, kind="ExternalInput")
    i128_d = nc.dram_tensor("i128", (P, P), bf16, kind="ExternalInput")
    out_d = nc.dram_tensor("partials", (P, 1), f32, kind="ExternalOutput")

    with tile.TileContext(nc) as tc:
        with (
            tc.tile_pool(name="big", bufs=1) as big,
            tc.tile_pool(name="work", bufs=2) as work,
            tc.tile_pool(name="psum", bufs=2, space=bass.MemorySpace.PSUM) as pp,
        ):
            # --- persistent SBUF tensors -------------------------------
            at_s = big.tile((P, KC, ROWS), f32)     # A slab, D-major, fp32
            at_b = big.tile((P, KC, ROWS), mmdt)    # A slab (matmul lhsT)
            atq_s = big.tile((P, KC, ROWS), bf16)   # A squared
            atsum_s = big.tile((P, ROWS), bf16)     # A sum-of-squares halves
            bt_b = big.tile((P, KC, M), mmdt)       # normalized B (rhs)
            dmask_s = big.tile((P, P), bf16)        # -1e9 * I  (mask weights)
            i128_s = big.tile((P, P), bf16)         # identity (mask rhs)
            ones_s = big.tile((P, P), bf16)
            sca_s = big.tile((P, NM), f32)          # 1/(|a|*T) per slab row
            lnit_s = big.tile((P, 1), f32)          # ln(1/T) bias vector
            acc_s = big.tile((P, NM * NST), f32)    # exp row-sums
            praw_s = big.tile((P, NM), f32)         # raw partner dots
            ssum_s = big.tile((P, NM), f32)
            lns_s = big.tile((P, NM), f32)
            lt_s = big.tile((P, NM), f32)
            outp_s = big.tile((P, 1), f32)

            # --- DMAs: B group 0 first, halved (it gates the pipeline
            # fill), then A, masks, then groups 1-3 whole.
            btf_tiles = [None] * NG

            def dma_group(g, split):
                gsl0 = g * GW
                btf = work.tile((P, KC, GW), f32, tag="btf", bufs=4)
                btf_tiles[g] = btf
                hw = GW // split
                for h in range(split):
                    for kc in range(KC):
                        nc.sync.dma_start(
                            btf[:, kc, h * hw : (h + 1) * hw],
                            bt_d.ap()[kc * P : (kc + 1) * P,
                                      gsl0 + h * hw : gsl0 + (h + 1) * hw])

            dma_group(0, split=2)
            nc.sync.dma_start(
                at_s[:], at_d.ap().rearrange("(k p) r -> p k r", p=P))
            nc.sync.dma_start(dmask_s[:], dmask_d.ap())
            nc.sync.dma_start(i128_s[:], i128_d.ap())
            for g in range(1, NG):
                dma_group(g, split=1)
            nc.vector.memset(ones_s[:], 1.0)
            nc.vector.memset(lnit_s[:], math.log(1.0 / TEMP))

            # Pre-load the ACT table set that holds BOTH Exp and Ln; the
            # fixpoint table-load pass then inserts no further reloads.
            from concourse.hw_specs import get_activation_tables
            _tabs = list(get_activation_tables(nc.m.arch))
            nc.scalar.add_instruction(
                mybir.InstLoadActFuncSet(
                    name=nc.scalar.bass.get_next_instruction_name(),
                    ins=[], outs=[],
                    act_func_set_id=_tabs.index("natural_log_exp_and_others")))

            # at_b row-chunk 0 on GpSimd (gates the very first logits
            # matmul); the bulk is cast on DVE after the fill drains.
            nc.gpsimd.tensor_copy(at_b[:, :, 0:P], at_s[:, :, 0:P])

            def colred(dst, src, nchunks):
                """dst[:, c] = sum over partitions of src[:, 128c..128c+128)
                via 1-column matmuls (distributed column norms)."""
                for c in range(nchunks):
                    nc.tensor.matmul(
                        dst[:, c : c + 1], src[:, c * P : (c + 1) * P],
                        ones_s[:, 0:1], start=True, stop=True,
                        skip_group_check=True)

            # --- group 0: full-width normalize (runs in the fill shadow
            # while ACT is idle), split in column halves to shorten the
            # critical chain.
            btf0 = btf_tiles[0]
            ssb0 = pp.tile((P, PSW), f32, tag="ps", bufs=2)
            invb0 = work.tile((P, GW), f32, tag="invbF", bufs=1)
            HW2 = GW // 2
            for h in range(2):
                hs = slice(h * HW2, (h + 1) * HW2)
                for kc in range(KC):
                    bsq = work.tile((P, HW2), bf16, tag="bsq0")
                    nc.vector.tensor_mul(bsq[:], btf0[:, kc, hs],
                                         btf0[:, kc, hs])
                    for c in range(HW2 // 512):
                        csl = slice(c * 512, (c + 1) * 512)
                        nc.tensor.matmul(
                            ssb0[:, h * HW2 : (h + 1) * HW2][:, csl],
                            ones_s[:], bsq[:, csl],
                            start=(kc == 0), stop=(kc == KC - 1),
                            skip_group_check=True)
            # A squared, on DVE between the squares and the nmuls
            nc.vector.tensor_mul(atq_s[:], at_s[:], at_s[:])
            nc.vector.tensor_add(atsum_s[:], atq_s[:, 0, :], atq_s[:, 1, :])
            for h in range(2):
                hs = slice(h * HW2, (h + 1) * HW2)
                nc.scalar.activation(invb0[:, hs], ssb0[:, hs], AF.Ln)
                nc.scalar.activation(invb0[:, hs], invb0[:, hs], AF.Exp,
                                     scale=-0.5)
                for kc in range(KC):
                    nc.vector.tensor_mul(bt_b[:, kc, hs], btf0[:, kc, hs],
                                         invb0[:, hs])

            # --- A row scales: column-reduce lands directly in sca's
            # (partition, m) layout; sca = Exp(-.5*Ln(ssa) + ln(1/T)).
            slota = pp.tile((P, PSW), f32, tag="ps", bufs=2)
            colred(slota, atsum_s[:], NM)
            nc.scalar.activation(sca_s[:], slota[:, 0:NM], AF.Ln)
            nc.scalar.activation(sca_s[:], sca_s[:], AF.Exp,
                                 scale=-0.5, bias=lnit_s[:])

            # bulk of the lhsT cast, after the fill-critical DVE work
            nc.vector.tensor_copy(at_b[:, :, P:ROWS], at_s[:, :, P:ROWS])

            def emit_norm_squares(g):
                """DVE squares + PE ones-matmul partition-reduce ->
                (128,2048) broadcast norms in a fresh PSUM ring slot."""
                btf = btf_tiles[g]
                slot = pp.tile((P, PSW), f32, tag="ps", bufs=2)
                for kc in range(KC):
                    bsq = work.tile((P, GW), bf16, tag="bsq")
                    nc.vector.tensor_mul(bsq[:], btf[:, kc, :],
                                         btf[:, kc, :])
                    for c in range(GW // 512):
                        csl = slice(c * 512, (c + 1) * 512)
                        nc.tensor.matmul(
                            slot[:, csl], ones_s[:], bsq[:, csl],
                            start=(kc == 0), stop=(kc == KC - 1),
                            skip_group_check=True)
                return slot

            def emit_norm_scale(g, slot):
                """Full-width rsqrt = Exp(-.5*Ln) on ACT (same table as
                the exps -> no reload); DVE applies it to B (fp8 out)."""
                btf = btf_tiles[g]
                osl = slice(g * GW, (g + 1) * GW)
                invb = work.tile((P, GW), f32, tag="invb")
                nc.scalar.activation(invb[:], slot[:, 0:NCH * P], AF.Ln)
                nc.scalar.activation(invb[:], invb[:], AF.Exp, scale=-0.5)
                for kc in range(KC):
                    nc.vector.tensor_mul(bt_b[:, kc, osl],
                                         btf[:, kc, :], invb[:])

            def emit_praw():
                """Positives: praw[p,m] = sum_d at[d, 128m+p] *
                bt_norm[d, 4096+128m+p] -- an elementwise product plus a
                column-reduce, landing directly in (partition, m) layout."""
                psl = slice(2 * GW, 2 * GW + ROWS)
                pq = work.tile((P, KC, ROWS), bf16, tag="pq", bufs=1)
                for kc in range(KC):
                    nc.vector.tensor_mul(pq[:, kc, :], at_s[:, kc, :],
                                         bt_b[:, kc, psl])
                pqs = work.tile((P, ROWS), bf16, tag="pqs", bufs=1)
                nc.vector.tensor_add(pqs[:], pq[:, 0, :], pq[:, 1, :])
                slotp = pp.tile((P, PSW), f32, tag="ps", bufs=2)
                colred(slotp, pqs[:], NM)
                nc.vector.tensor_copy(praw_s[:], slotp[:, 0:NM])

            # --- main loop: logits + exp for group g; group g+1's
            # normalize is emitted inside group g's loop so everything
            # overlaps and ring slots free quickly.
            pend = None
            for g in range(NG):
                for m in range(NM):
                    if g + 1 < NG:
                        if m == 1:
                            pend = emit_norm_squares(g + 1)
                        elif m == 2:
                            emit_norm_scale(g + 1, pend)
                    if g == 2 and m == 6:
                        # bt_b group 2 has been normalized since last
                        # group; extract the positives off to the side
                        emit_praw()
                    lg = pp.tile((P, PSW), f32, tag="ps", bufs=2)
                    dchunk = (m * P) // 512  # chunk holding the diagonal
                    if USE_FP8:
                        for c in range(PSW // 512):
                            csl = slice(c * 512, (c + 1) * 512)
                            bsl = slice(g * GW + c * 512,
                                        g * GW + (c + 1) * 512)
                            nc.tensor.matmul(
                                lg[:, csl],
                                at_b[:, :, m * P : (m + 1) * P],
                                bt_b[:, :, bsl],
                                start=True,
                                stop=(g != 0 or c != dchunk),
                                skip_group_check=True,
                                perf_mode=mybir.MatmulPerfMode.DoubleRow)
                    else:
                        for kc in range(KC):
                            for c in range(PSW // 512):
                                csl = slice(c * 512, (c + 1) * 512)
                                bsl = slice(g * GW + c * 512,
                                            g * GW + (c + 1) * 512)
                                nc.tensor.matmul(
                                    lg[:, csl],
                                    at_b[:, kc, m * P : (m + 1) * P],
                                    bt_b[:, kc, bsl],
                                    start=(kc == 0),
                                    stop=(kc == KC - 1)
                                    and (g != 0 or c != dchunk),
                                    skip_group_check=True)
                    if g == 0:
                        # diagonal -1e9: one more matmul into the same
                        # accumulation group
                        msl = slice(m * P, m * P + P)
                        nc.tensor.matmul(
                            lg[:, msl], dmask_s[:], i128_s[:],
                            start=False, stop=True, skip_group_check=True)
                    ai = m * NST + g
                    # exp in place in PSUM; row-sum via the ACT accumulator
                    nc.scalar.activation(
                        lg[:], lg[:], AF.Exp,
                        bias=0.0, scale=sca_s[:, m : m + 1],
                        accum_out=acc_s[:, ai : ai + 1])

            # --- assembly: loss rows = ln(S) - praw * sca --------------
            for m in range(NM):
                nc.vector.reduce_sum(
                    ssum_s[:, m : m + 1], acc_s[:, m * NST : (m + 1) * NST],
                    axis=AX.X)
            nc.scalar.activation(lns_s[:], ssum_s[:], AF.Ln)
            nc.vector.tensor_mul(praw_s[:], praw_s[:], sca_s[:])
            nc.vector.tensor_sub(lt_s[:], lns_s[:], praw_s[:])
            nc.vector.reduce_sum(outp_s[:], lt_s[:], axis=AX.X)
            nc.sync.dma_start(out_d.ap(), outp_s[:])

    nc.compile()
    return nc


def get_nc():
    if "nc" not in _CACHE:
        _CACHE["nc"] = _build_nc()
    return _CACHE["nc"]


def make_in_maps(A: np.ndarray, B: np.ndarray) -> list[dict]:
    import ml_dtypes

    A = np.asarray(A, dtype=np.float32)
    B = np.asarray(B, dtype=np.float32)
    # view-major D-major matrices: X[d, v*N + n] = X_in[n, v, d]
    At = np.ascontiguousarray(A.transpose(2, 1, 0).reshape(D, M))
    Bt = np.ascontiguousarray(B.transpose(2, 1, 0).reshape(D, M))
    dmask = np.zeros((P, P), dtype=np.float32)
    np.fill_diagonal(dmask, NEG)
    dmask = dmask.astype(ml_dtypes.bfloat16)
    i128 = np.eye(P, dtype=np.float32).astype(ml_dtypes.bfloat16)
    in_maps = []
    for k in range(NCORES):
        at_k = np.ascontiguousarray(At[:, k * ROWS : (k + 1) * ROWS])
        # rotate columns so local col j holds global col (j + 1024k) % 8192
        bt_k = np.ascontiguousarray(np.roll(Bt, -ROWS * k, axis=1))
        in_maps.append({"at": at_k, "bt": bt_k,
                        "dmask": dmask, "i128": i128})
    return in_maps


def kernel(A: np.ndarray, B: np.ndarray) -> np.ndarray:
    from concourse.bass_utils import run_bass_kernel_spmd

    in_maps = make_in_maps(A, B)
    nc = get_nc()
    trace = bool(int(os.environ.get("KERNEL_TRACE", "0")))
    res = run_bass_kernel_spmd(
        nc, in_maps, core_ids=list(range(NCORES)), trace=trace)
    total = 0.0
    for r in res.results:
        total += float(r["partials"].astype(np.float64).sum())
    if res.exec_time_ns is not None:
        print(f"[kernel] exec_time_ns={res.exec_time_ns}")
        _CACHE["exec_time_ns"] = res.exec_time_ns
    _CACHE["last_results"] = res
    return np.float32(total / M)
